# revision 4
# baseline (speedup 1.0000x reference)
"""Trainium2 Bass kernel for nn_Attention_65343632441735 (XCA-style channel
attention: 1x1 conv -> depthwise 3x3 -> channel attention -> 1x1 proj).

Sharding: data-parallel over batch (8 images, 1 per NeuronCore).

Single-core schedule (v2):
- oc chunking [128,128,128,96,96]: the two v head-pairs are standalone
  96-wide chunks, so no partition-shift DMAs or vshift copies.
- bf16 inputs/outputs on the wire (host casts); all DMA issue on the SP
  (sync) engine so no compute engine is held by transfers.
- depthwise 3x3 split by measured engine cost: DVE runs chunk 0 + vB as
  TS-mul trees (4x fast mode) with wide pairwise adds (2x); Pool runs
  chunk 2 as a TS-mul + TT-add chain; PE runs chunks 1 and 3 (vA) as
  diag-matmul accumulations; two vB taps ride on ACT. The last slab
  swaps chunk2<->vB so the tail starts sooner.
- gram for slab s-1 issues at the top of slab s to keep PE's in-order
  stream dense.
- tail: rkb row-broadcast via ones-matmul on PE (no DRAM round-trip);
  softmax skips max-subtraction (pre-softmax values bounded by
  temperature; masked entries underflow exp to exactly 0).
- stage C: 2 px-tiles per output DMA, copies split DVE/ACT, outsb
  triple-buffered.
- conv halo reuse: each slab convolves only its 16 new rows (loads and
  x+f adds also skip the 2 overlap rows); the overlap rows are copied
  from the previous slab's pre buffer on ACT.
- x+f adds for slab s+1 are issued mid-slab s, and gram(s-1) issues at
  the top of slab s, keeping the in-order engine streams dense.
"""

import numpy as np
import ml_dtypes

import concourse.bass as bass
import concourse.tile as tile
from concourse import mybir
from concourse.bass_utils import run_bass_kernel_spmd

F32 = mybir.dt.float32
BF16 = mybir.dt.bfloat16
AL = mybir.AluOpType
ACTF = mybir.ActivationFunctionType

C = 192          # input channels
OC = 576         # 3*C qkv channels
HEADS = 4
CH = 48          # channels per head
W = 128          # image width (one row = one 128-partition chunk)
EPS = 1e-12

# oc chunking: 3 full 128-chunks (q,k) + two 96-chunks (v pair A, v pair B)
OCW = [128, 128, 128, 96, 96]
OCB = [0, 128, 256, 384, 480]
DVE_CHUNKS = (0, 1)     # q/k channels: DVE TS-mul tree
POOL_QK_CHUNK = 2       # q/k channels: Pool fused STT
PE_V_CHUNK = 3          # vA: diag matmuls on PE
POOL_V_CHUNK = 4        # vB: Pool fused STT (f32 accum + fused cast)
TAPS = [(di, dj) for di in (-1, 0, 1) for dj in (-1, 0, 1)]


def _bf(a):
    return np.ascontiguousarray(a.astype(ml_dtypes.bfloat16))


def prep_weights(w_qkv, w_dw, w_proj, temperature):
    wqkvT = _bf(w_qkv[:, :, 0, 0].T)                       # [192, 576]
    dwv = np.zeros((128, 5, 9), np.float32)                # per-partition taps
    for m in range(5):
        ow = OCW[m]
        b = OCB[m]
        for t in range(9):
            di, dj = TAPS[t]
            dwv[:ow, m, t] = w_dw[b:b + ow, 0, di + 1, dj + 1]
    # diag mats for PE chunks: ci 0,1 -> chunks 1,3
    dgm = np.zeros((128, 2, 9, 128), np.float32)
    for ci, (b, ow) in enumerate(((128, 128), (384, 96))):
        for t in range(9):
            di, dj = TAPS[t]
            np.fill_diagonal(dgm[:ow, ci, t, :ow],
                             w_dw[b:b + ow, 0, di + 1, dj + 1])
    eye96 = np.eye(96, dtype=np.float32)
    ones96 = np.ones((1, 96), np.float32)
    # additive mask: 0 on the two 48x48 diagonal blocks, -1e30 off-diagonal
    blkmask = np.full((96, 96), -1e30, np.float32)
    blkmask[0:48, 0:48] = 0.0
    blkmask[48:96, 48:96] = 0.0
    # wproj rows grouped by head-pair: wpjp[c, p, o] = wprojT[96p + c, o]
    wpjp = _bf(w_proj[:, :, 0, 0].T.reshape(2, 96, C).transpose(1, 0, 2))
    # temperature per head-pair block: temps96[r, p] = temperature[2p + r//48]
    t = temperature.reshape(HEADS)
    temps96 = np.zeros((96, 2), np.float32)
    for p in range(2):
        temps96[0:48, p] = t[2 * p]
        temps96[48:96, p] = t[2 * p + 1]
    return {
        "wqkvT": wqkvT, "dwv": dwv, "dgm": _bf(dgm),
        "eye96": eye96, "ones96": ones96, "wpjp": wpjp, "temps96": temps96,
        "blkmask": blkmask,
    }


def build_nc(H=128, legalize=True):
    assert H % 16 == 0
    NS = H // 16            # slabs of 16 rows
    HW = H * W
    NPT = HW // 512         # 512-px tiles for output stage

    nc = bass.Bass("TRN2")
    x_d = nc.dram_tensor("x", (C, H, W), BF16, kind="ExternalInput")
    f_d = nc.dram_tensor("f", (C, H, W), BF16, kind="ExternalInput")
    wqkvT_d = nc.dram_tensor("wqkvT", (C, OC), BF16, kind="ExternalInput")
    wpjp_d = nc.dram_tensor("wpjp", (96, 2, C), BF16, kind="ExternalInput")
    dwv_d = nc.dram_tensor("dwv", (128, 5, 9), F32, kind="ExternalInput")
    dgm_d = nc.dram_tensor("dgm", (128, 2, 9, 128), BF16, kind="ExternalInput")
    eye_d = nc.dram_tensor("eye96", (96, 96), F32, kind="ExternalInput")
    ones_d = nc.dram_tensor("ones96", (1, 96), F32, kind="ExternalInput")
    msk_d = nc.dram_tensor("blkmask", (96, 96), F32, kind="ExternalInput")
    tmp_d = nc.dram_tensor("temps96", (96, 2), F32, kind="ExternalInput")
    out_d = nc.dram_tensor("out", (C, H, W), BF16, kind="ExternalOutput")

    with tile.TileContext(nc) as tc:
        _body(nc, tc, H, NS, HW, NPT, x_d, f_d, wqkvT_d, wpjp_d, dwv_d,
              dgm_d, eye_d, ones_d, msk_d, tmp_d, out_d)
    nc.finalize()
    if legalize:
        legalize_waits(nc)
    return nc


def _body(nc, tc, H, NS, HW, NPT, x_d, f_d, wqkvT_d, wpjp_d, dwv_d, dgm_d,
          eye_d, ones_d, msk_d, tmp_d, out_d):
    import contextlib
    ctx = contextlib.ExitStack()
    with ctx:
        const = ctx.enter_context(tc.tile_pool(name="const", bufs=1))
        xin_p = ctx.enter_context(tc.tile_pool(name="xin", bufs=2))
        xf_p = ctx.enter_context(tc.tile_pool(name="xf", bufs=1))
        pre_p = ctx.enter_context(tc.tile_pool(name="pre", bufs=2))
        qkdw_p = ctx.enter_context(tc.tile_pool(name="qkdw", bufs=1))
        qkT_p = ctx.enter_context(tc.tile_pool(name="qkT", bufs=1))
        tmpt_p = ctx.enter_context(tc.tile_pool(name="tmpt", bufs=1))
        vbuf_p = ctx.enter_context(tc.tile_pool(name="vbuf", bufs=1))
        tail_p = ctx.enter_context(tc.tile_pool(name="tail", bufs=1))
        outsb_p = ctx.enter_context(tc.tile_pool(name="outsb", bufs=3))
        ps_p = ctx.enter_context(tc.tile_pool(name="ps", bufs=4, space="PSUM"))
        psg_p = ctx.enter_context(tc.tile_pool(name="psg", bufs=1, space="PSUM"))
        pst_p = ctx.enter_context(tc.tile_pool(name="pst", bufs=1, space="PSUM"))

        # ---- constants (stage-A-critical first; tail-only consts load on
        # the scalar engine so SP can start the first input loads at once) ----
        wq1 = const.tile([128, OC], BF16)
        wq2 = const.tile([64, OC], BF16)
        nc.sync.dma_start(wq1[:], wqkvT_d[0:128, :])
        nc.sync.dma_start(wq2[:], wqkvT_d[128:192, :])
        dwv = const.tile([128, 5, 9], F32)
        nc.scalar.dma_start(dwv[:], dwv_d[:])
        dgm = const.tile([128, 2, 9, 128], BF16)
        nc.scalar.dma_start(dgm[:], dgm_d[:])
        wpj = const.tile([96, 2, C], BF16)
        nc.scalar.dma_start(wpj[:], wpjp_d[:])
        eye = const.tile([96, 96], F32)
        nc.scalar.dma_start(eye[:], eye_d[:])
        ones96 = const.tile([1, 96], F32)
        nc.scalar.dma_start(ones96[:], ones_d[:])
        msk = const.tile([96, 96], F32)
        nc.scalar.dma_start(msk[:], msk_d[:])
        tmps = const.tile([96, 2], F32)
        nc.scalar.dma_start(tmps[:], tmp_d[:])

        vA = vbuf_p.tile([96, HW], BF16)   # v pair A (heads 0,1), oc384-479
        vB = vbuf_p.tile([96, HW], BF16)   # v pair B (heads 2,3), oc480-575
        # Gp[p][:, 0, :] = q_pair @ k_pair.T; [:,1,:] = q@q.T; [:,2,:] = k@k.T
        Gp = [psg_p.tile([96, 3, 96], F32, tag=f"G{p}", name=f"G{p}")
              for p in range(2)]

        # ================= stage A: conv + depthwise + gram =================
        # gram for slab s-1 is issued at the top of slab s so PE's in-order
        # stream never stalls on slab s's transposes (gram(s-1) is ready).
        prev_qkT = None

        def gram(qkT_t, s_):
            for pc in range(16):
                st = (s_ == 0 and pc == 0)
                sp = (s_ == NS - 1 and pc == 15)
                for p in range(2):
                    qs = qkT_t[:, pc, 96 * p:96 * p + 96]
                    ks = qkT_t[:, pc, 192 + 96 * p:192 + 96 * p + 96]
                    G = Gp[p]
                    nc.tensor.matmul(G[:, 0, :], qs, ks, start=st, stop=sp,
                                     skip_group_check=True)
                    nc.tensor.matmul(G[:, 1, :], qs, qs, start=st, stop=sp,
                                     skip_group_check=True)
                    nc.tensor.matmul(G[:, 2, :], ks, ks, start=st, stop=sp,
                                     skip_group_check=True)

        def load_slab(s):
            r0 = 16 * s - 1
            rs, re = max(r0, 0), min(16 * s + 17, H)
            nrows = re - rs
            ro = rs - r0  # offset of first loaded row inside 18-row window
            if s > 0:
                # conv only consumes window rows >= 2 (rows 0,1 come from the
                # previous slab's pre): skip loading/adding the overlap
                rs += 2 - ro
                ro = 2
                nrows = re - rs
            xin1 = xin_p.tile([128, 18, W], BF16, tag="xin1")
            xin2 = xin_p.tile([64, 18, W], BF16, tag="xin2")
            xf1 = xf_p.tile([128, 18, W], BF16, tag="xf1")
            xf2 = xf_p.tile([64, 18, W], BF16, tag="xf2")
            nc.sync.dma_start(xin1[:, ro:ro + nrows, :], x_d[0:128, rs:re, :])
            nc.sync.dma_start(xf1[:, ro:ro + nrows, :], f_d[0:128, rs:re, :])
            nc.sync.dma_start(xin2[:, ro:ro + nrows, :], x_d[128:C, rs:re, :])
            nc.sync.dma_start(xf2[:, ro:ro + nrows, :], f_d[128:C, rs:re, :])
            return xin1, xin2, xf1, xf2, ro, nrows

        def add_slab(t):
            xin1, xin2, xf1, xf2, ro, nrows = t
            nc.vector.tensor_add(xin1[:, ro:ro + nrows, :],
                                 xin1[:, ro:ro + nrows, :],
                                 xf1[:, ro:ro + nrows, :])
            nc.vector.tensor_add(xin2[:, ro:ro + nrows, :],
                                 xin2[:, ro:ro + nrows, :],
                                 xf2[:, ro:ro + nrows, :])

        cur = load_slab(0)
        add_slab(cur)
        for s in range(NS):
            if prev_qkT is not None:
                gram(prev_qkT, s - 1)
            xin1, xin2, _, _, ro, nrows = cur
            nxt = load_slab(s + 1) if s + 1 < NS else None

            pre = pre_p.tile([128, 5, 18, 130], BF16, tag="pre")
            nc.vector.memset(pre[:, :, :, 0:1], 0.0)
            nc.vector.memset(pre[:, :, :, 129:130], 0.0)
            if s == 0:
                nc.vector.memset(pre[:, :, 0, :], 0.0)
            if s == NS - 1:
                nc.vector.memset(pre[:, :, 17, :], 0.0)

            # 1x1 conv: qkv_pre[oc, px] = wqkvT.T @ x_in, px tiles of 4 rows
            row_tiles = []
            rr = ro
            while rr < ro + nrows:
                rw = min(4, ro + nrows - rr)
                row_tiles.append((rr, rw))
                rr += rw
            for m in range(5):
                ow = OCW[m]
                b = OCB[m]
                for (rt, rw) in row_tiles:
                    ptw = rw * W
                    acc = ps_p.tile([128, 512], F32, tag="psA")
                    nc.tensor.matmul(
                        acc[0:ow, 0:ptw],
                        wq1[:, b:b + ow],
                        xin1[:, rt:rt + rw, :],
                        start=True, stop=False)
                    nc.tensor.matmul(
                        acc[0:ow, 0:ptw],
                        wq2[:, b:b + ow],
                        xin2[:, rt:rt + rw, :],
                        start=False, stop=True)
                    nc.scalar.copy(
                        pre[0:ow, m, rt:rt + rw, 1:1 + W],
                        acc[0:ow, 0:ptw])

            def pre_view(m, di, dj, ow, rbase=1, nr=16):
                return pre[0:ow, m, rbase + di:rbase + di + nr,
                           1 + dj:1 + dj + W]

            qkdw = qkdw_p.tile([128, 3, 16, W], BF16, tag="qkdw")
            tmpt = tmpt_p.tile([128, 4, 16, W], BF16, tag="tmpt")

            def dve_tree(m, r0, nr, dst_ap=None, act_taps=()):
                """dst rows [r0, r0+nr) of chunk m via TS-mul tree on DVE.
                Taps in act_taps (subset of 7,8) compute on ACT instead."""
                ow = OCW[m]
                dst = (qkdw[0:ow, m, r0:r0 + nr, :] if dst_ap is None
                       else dst_ap)

                def pv(t):
                    di, dj = TAPS[t]
                    return pre_view(m, di, dj, ow, rbase=1 + r0, nr=nr)

                def ts(t, slot):
                    if t in act_taps:
                        nc.scalar.mul(tmpt[0:ow, slot, 0:nr, :], pv(t),
                                      dwv[0:ow, m, t:t + 1])
                    else:
                        nc.vector.tensor_scalar_mul(
                            tmpt[0:ow, slot, 0:nr, :], pv(t),
                            dwv[0:ow, m, t:t + 1])

                # t7,t8 go to dedicated slots 4,5 (ACT-issued first when
                # offloaded so they overlap DVE's own taps)
                for t in act_taps:
                    ts(t, t - 3)
                nc.vector.tensor_scalar_mul(dst, pv(0), dwv[0:ow, m, 0:1])
                for t in (1, 2, 3, 4):
                    ts(t, t - 1)
                nc.vector.tensor_add(
                    tmpt[0:ow, 0:2, 0:nr, :], tmpt[0:ow, 0:2, 0:nr, :],
                    tmpt[0:ow, 2:4, 0:nr, :])
                for t in (5, 6):
                    ts(t, t - 3)
                nc.vector.tensor_add(
                    tmpt[0:ow, 0:2, 0:nr, :], tmpt[0:ow, 0:2, 0:nr, :],
                    tmpt[0:ow, 2:4, 0:nr, :])
                for t in (7, 8):
                    ts(t, t - 5)
                nc.vector.tensor_add(
                    tmpt[0:ow, 0:2, 0:nr, :], tmpt[0:ow, 0:2, 0:nr, :],
                    tmpt[0:ow, 2:4, 0:nr, :])
                nc.vector.tensor_add(
                    tmpt[0:ow, 0, 0:nr, :], tmpt[0:ow, 0, 0:nr, :],
                    tmpt[0:ow, 1, 0:nr, :])
                nc.vector.tensor_add(dst, dst, tmpt[0:ow, 0, 0:nr, :])

            def pe_diag(m, ci, pt_list, dst_fn):
                """chunk m depthwise px-tiles via diag matmuls on PE."""
                ow = OCW[m]
                for pt in pt_list:
                    acc = ps_p.tile([128, 512], F32, tag="psA")
                    for t, (di, dj) in enumerate(TAPS):
                        nc.tensor.matmul(
                            acc[0:ow, :],
                            dgm[0:ow, ci, t, 0:ow],
                            pre_view(m, di, dj, ow, rbase=1 + 4 * pt, nr=4),
                            start=(t == 0), stop=(t == 8))
                    dst_fn(pt, acc)

            # chunk 0: DVE tree; chunks 1, 3 (vA): PE diag matmuls
            dve_tree(0, 0, 16)
            # x+f adds for the NEXT slab here, so slab s+1's conv -> pre
            # copies are ready before its DVE taps need them
            if nxt is not None:
                add_slab(nxt)
            pe_diag(1, 0, range(3), lambda pt, acc: nc.scalar.copy(
                qkdw[:, 1, 4 * pt:4 * pt + 4, :], acc[:, :]))
            dve_tree(1, 12, 4)
            pe_diag(PE_V_CHUNK, 1, range(4), lambda pt, acc: nc.scalar.copy(
                vA[0:96, 2048 * s + 512 * pt:2048 * s + 512 * pt + 512],
                acc[0:96, :]))

            def pool_chain(m, dst):
                # self-contained TS-mul + TT-add chain on Pool
                ow = OCW[m]
                poolt = tmpt_p.tile([128, 2, 16, W], BF16, tag="poolt")
                nc.gpsimd.tensor_scalar_mul(dst, pre_view(m, -1, -1, ow),
                                            dwv[0:ow, m, 0:1])
                for t in range(1, 9):
                    di, dj = TAPS[t]
                    pslot = poolt[0:ow, t % 2, :, :]
                    nc.gpsimd.tensor_scalar_mul(pslot, pre_view(m, di, dj, ow),
                                                dwv[0:ow, m, t:t + 1])
                    nc.gpsimd.tensor_add(dst, dst, pslot)

            # vB (chunk 4) slab view
            m = POOL_V_CHUNK
            px0 = 2048 * s
            vslab = vB[0:96, px0:px0 + 2048]
            vslab_v = bass.AP(tensor=vslab.tensor, offset=vslab.offset,
                              ap=[[vslab.ap[0][0], 96], [W, 16], [1, W]])
            if s < NS - 1:
                pool_chain(POOL_QK_CHUNK, qkdw[0:128, POOL_QK_CHUNK, :, :])
                dve_tree(m, 0, 16, dst_ap=vslab_v)
            else:
                # last slab: chunk2 on DVE so the transpose/gram/tail start
                # sooner; vB's Pool chain hides under the tail (stage C is
                # its only consumer)
                dve_tree(POOL_QK_CHUNK, 0, 16)
                pool_chain(m, vslab_v)

            # ---- transpose q,k slab -> [px, ch] layout ----
            qkT = qkT_p.tile([128, 16, 384], BF16, tag="qkT")
            for m in range(3):
                nc.sync.dma_start_transpose(
                    qkT[:, :, 128 * m:128 * (m + 1)],
                    qkdw[:, m, :, :])
            prev_qkT = qkT
            cur = nxt

        gram(prev_qkT, NS - 1)

        # ================= attention tail =================
        scr = tail_p.tile([96, 96], F32)
        # sqall columns: [qq_p0, qq_p1, kk_p0, kk_p1]
        sqall = tail_p.tile([96, 4], F32)
        for p in range(2):
            nc.vector.tensor_mul(scr[:], Gp[p][:, 1, :], eye[:])
            nc.vector.tensor_reduce(sqall[:, p:p + 1], scr[:],
                                    axis=mybir.AxisListType.X, op=AL.add)
            nc.vector.tensor_mul(scr[:], Gp[p][:, 2, :], eye[:])
            nc.vector.tensor_reduce(sqall[:, 2 + p:3 + p], scr[:],
                                    axis=mybir.AxisListType.X, op=AL.add)
        nrm = tail_p.tile([96, 4], F32)
        nc.scalar.activation(nrm[:], sqall[:], ACTF.Sqrt)
        nc.vector.tensor_scalar_max(nrm[:], nrm[:], EPS)
        rn = tail_p.tile([96, 4], F32)
        nc.vector.reciprocal(rn[:], nrm[:])
        MpT = tail_p.tile([96, 2, C], BF16)
        for p in range(2):
            at = tail_p.tile([96, 96], F32, tag=f"at{p}")
            nc.vector.tensor_scalar_mul(at[:], Gp[p][:, 0, :], rn[:, p:p + 1])
            # k-norm reciprocals along the free dim:
            # [96,1] -T-> [1,96] -> broadcast to [96,96] via ones-matmul on PE
            rT_ps = pst_p.tile([1, 96], F32, tag="pstail")
            nc.tensor.transpose(rT_ps[:], rn[:, 2 + p:3 + p], eye[:])
            rT = tail_p.tile([1, 96], F32, tag=f"rT{p}")
            nc.vector.tensor_copy(rT[:], rT_ps[:])
            rkb_ps = pst_p.tile([96, 96], F32, tag="pstail")
            nc.tensor.matmul(rkb_ps[:], ones96[:], rT[:], start=True,
                             stop=True, skip_group_check=True)
            nc.vector.tensor_mul(at[:], at[:], rkb_ps[:])
            nc.vector.tensor_add(at[:], at[:], msk[:])
            mx = tail_p.tile([96, 1], F32, tag=f"mx{p}")
            nc.vector.tensor_reduce(mx[:], at[:], axis=mybir.AxisListType.X,
                                    op=AL.max)
            mb = tail_p.tile([96, 1], F32, tag=f"mb{p}")
            nc.vector.tensor_scalar(out=mb[:], in0=mx[:],
                                    scalar1=tmps[:, p:p + 1], scalar2=-1.0,
                                    op0=AL.mult, op1=AL.mult)
            ae = tail_p.tile([96, 96], F32, tag=f"ae{p}")
            se = tail_p.tile([96, 1], F32, tag=f"se{p}")
            nc.scalar.activation(out=ae[:], in_=at[:], func=ACTF.Exp,
                                 bias=mb[:], scale=tmps[:, p:p + 1],
                                 accum_out=se[:])
            rs_ = tail_p.tile([96, 1], F32, tag=f"rs{p}")
            nc.vector.reciprocal(rs_[:], se[:])
            abp = tail_p.tile([96, 96], BF16, tag=f"abp{p}")
            nc.vector.tensor_scalar_mul(abp[:], ae[:], rs_[:])
            # MpT[d, o] = sum_c abp[c, d] * wpjp[c, p, o]  (block-diag abp)
            mh_ps = pst_p.tile([96, C], F32, tag="pstail")
            nc.tensor.matmul(mh_ps[:], abp[:], wpj[:, p, :], start=True,
                             stop=True, skip_group_check=True)
            nc.vector.tensor_copy(MpT[:, p, :], mh_ps[:])

        # ========== stage C: out = sum_p MpT_p.T @ v_pair_p, then DMA ======
        # 2 px-tiles per output DMA; copies split DVE/ACT; DMAs on SP
        for g in range(NPT // 2):
            for mc, (o0, ow) in enumerate(((0, 128), (128, 64))):
                osb = outsb_p.tile([128, 2, 512], BF16, tag="osb")
                for j in range(2):
                    nt = 2 * g + j
                    c0 = 512 * nt
                    acc = ps_p.tile([128, 512], F32, tag="psA")
                    nc.tensor.matmul(acc[0:ow, :], MpT[:, 0, o0:o0 + ow],
                                     vA[:, c0:c0 + 512], start=True,
                                     stop=False, skip_group_check=True)
                    nc.tensor.matmul(acc[0:ow, :], MpT[:, 1, o0:o0 + ow],
                                     vB[:, c0:c0 + 512], start=False,
                                     stop=True, skip_group_check=True)
                    if mc == 0:
                        nc.vector.tensor_copy(osb[0:ow, j, :], acc[0:ow, :])
                    else:
                        nc.scalar.copy(osb[0:ow, j, :], acc[0:ow, :])
                nc.sync.dma_start(out_d[o0:o0 + ow, 8 * g:8 * g + 8, :],
                                  osb[0:ow, :, :])


def legalize_waits(nc):
    """This walrus build encodes at most ONE sync-wait per instruction (none on
    Drain): hoist extras onto injected single-wait NoOps."""
    n_fix = 0
    for fn in nc.m.functions:
        for bb in fn.blocks:
            insts = list(bb.instructions)
            new_insts = []
            changed = False
            for ins in insts:
                si = ins.sync_info
                waits = list(si.on_wait) if si is not None else []
                keep = 0 if type(ins).__name__ == "InstDrain" else 1
                if len(waits) > keep:
                    n_hoist = len(waits) - keep
                    hoisted, kept = waits[:n_hoist], waits[n_hoist:]
                    for j, w in enumerate(hoisted):
                        new_insts.append(mybir.InstNoOp(
                            name=f"{ins.name}_hw{j}", engine=ins.engine,
                            sync_info=mybir.SyncInfo(on_wait=[w], on_update=[]),
                            bass_nofuse=True))
                        n_fix += 1
                    ins.sync_info = mybir.SyncInfo(
                        on_wait=kept, on_update=list(si.on_update) if si else [])
                    changed = True
                new_insts.append(ins)
            if changed:
                try:
                    bb.instructions = new_insts
                except Exception:
                    bb.instructions.clear()
                    bb.instructions.extend(new_insts)
    return n_fix


_NC_CACHE = {}


def _get_nc(H):
    if H not in _NC_CACHE:
        _NC_CACHE[H] = build_nc(H)
    return _NC_CACHE[H]


def kernel(x, f, w_qkv, w_dw, w_proj, temperature, _H=None, _trace=False):
    x = np.asarray(x, np.float32)
    f = np.asarray(f, np.float32)
    b = x.shape[0]
    H = x.shape[2] if _H is None else _H
    wts = prep_weights(np.asarray(w_qkv, np.float32),
                       np.asarray(w_dw, np.float32),
                       np.asarray(w_proj, np.float32),
                       np.asarray(temperature, np.float32))
    nc = _get_nc(H)
    xb = _bf(x)
    fb = _bf(f)
    in_maps = []
    for i in range(b):
        m = {"x": np.ascontiguousarray(xb[i]),
             "f": np.ascontiguousarray(fb[i])}
        m.update(wts)
        in_maps.append(m)
    res = run_bass_kernel_spmd(nc, in_maps, core_ids=list(range(b)),
                               trace=_trace)
    out = np.stack([res.results[i]["out"].astype(np.float32)
                    for i in range(b)], axis=0)
    kernel.last_results = res
    return out
